# revision 25
# baseline (speedup 1.0000x reference)
"""NT-Xent loss on 8 Trainium2 NeuronCores — fp8 + symmetric 5/8 scheme.

Math: z = concat(z_i, z_j) [8192, 256]; E = exp(2*cos_sim(z)) is
symmetric, so row sums = column sums. Blocked over 8 superblocks of
1024 rows, core c owns rows [1024c, 1024c+1024) and computes sim
against column superblocks d = 0..4 (its own + the next four, mod 8).
d in {1,2,3} blocks are mirrored via COLUMN sums (row sums of the
uncomputed transpose block); the d=4 block is computed by both members
of each (c, c+4) pair, so no mirror is needed there. Every global row
then receives exactly 8 superblock contributions: 5 direct row-sum
partials from its owner core + 3 column-sum partials from cores
c-1, c-2, c-3. The host assembles denom = sum - e^2, takes ln in
fp64, and averages — O(2N) work.

Per core (identical SPMD program on column-rotated inputs):
  - Host normalizes rows in fp64, scales by 16, quantizes to fp8 e4m3,
    ships the transposed K-packed layout znt[p, h, j] = q[j, h*128+p]
    ([128, 2, 5120], rotated by -1024c cols: local cols 0:5120 =
    superblocks c..c+4, local rows = local cols 0:1024).
  - Sim matmuls: fp8 DoubleRow, one matmul per [128, 512] output
    covering the full K=256 contraction. PSUM = 256*sim.
  - ACT exps [128, 2048] PSUM windows (scale 1/128) with fused row-sum
    accumulation, writing exp as bf16 to SBUF (esb) for the colsum
    path. FIVE of the 16 q-windows are offloaded to DVE via the
    Schraudolph bit-trick exp (int32(A*x+B) then read the bits as f32;
    ~3% per-element error that washes out in the row sums), balancing
    ACT (the pacer) against DVE idle time. The [4096:5120) d=4 window
    is exp'd in place on ACT and carries the positive-pair diagonal
    (extracted pre-exp by a DVE identity-mask reduce).
  - Colsums over cols 1024:4096: incremental DVE bf16 tree-sum over
    the 8 m-chunks (2x mode), then one ones-matmul per 512-col block
    -> [1, 512] PSUM -> DVE copy -> colsum_sb, spread across the d4
    phase.
  - Outputs per core: rowsums [128, 8], colsums [1, 3072], posv
    [128, 8] (scaled sim, x1/128 on host).

Measured pitfalls baked in: DVE tensor_reduce has no 2x uop (use ACT
accum_out for rowsums); gpsimd partition_all_reduce is ~5x slower than
its cost model (use ones-matmuls); fp8 DoubleRow streams ~0.5-0.6
ns/moving-row (not the 2x the cost model claims); ACT costs ~0.95
ns/col on these windows regardless of destination.

fp8 error budget: e4m3 sigma~3.6%/elem -> sim noise sigma~3e-3 which
averages out in row sums and the 8192-row mean; Schraudolph adds ~3%
per-element on 5/16 of the denominator mass -> ~1e-4 loss error.
"""

import os
import sys

sys.path.insert(0, "/opt/trn_rl_repo")
os.environ.setdefault("MYCRO_LOCAL_CACHE", "1")

import numpy as np

import concourse.bass as bass
import concourse.mybir as mybir
from concourse import bacc, tile
from concourse.bass_utils import run_bass_kernel_spmd

F32 = mybir.dt.float32
BF16 = mybir.dt.bfloat16
FP8 = mybir.dt.float8e4
I32 = mybir.dt.int32
AF = mybir.ActivationFunctionType
ALU = mybir.AluOpType
DROW = mybir.MatmulPerfMode.DoubleRow

N_CORES = 8
TWO_N = 8192
D = 256
P = 128
ROWS_PER_CORE = TWO_N // N_CORES  # 1024
M_CHUNKS = ROWS_PER_CORE // P     # 8 local row chunks
NCOL = 512                        # matmul free dim (one PSUM bank)
QCOL = 2048                       # ACT window for q=0,1
LCOLS = 5 * ROWS_PER_CORE         # 5120 local cols (superblocks d=0..4)
CS_LO, CS_HI = 1024, 4096         # colsum region (d=1..3)
POS_Q0 = 4096                     # start of the d=4 window
N_WIN = 3                         # rowsum slots per m: q0, q1, d4
TEMP_SCALE = 2.0
QSCALE = 16.0                     # host fp8 quant scale; psum = 256*sim
ACT_SCALE = TEMP_SCALE / (QSCALE * QSCALE)   # exp(psum/128)
POS_SCALE = TEMP_SCALE / (QSCALE * QSCALE)

# Schraudolph: exp(x) ~ bits_as_f32(int32(2^23/ln2 * x + 127*2^23 - C))
SCH_A = ACT_SCALE * (1 << 23) / float(np.log(2.0))   # applied to psum
SCH_B = float(127 * (1 << 23) - 366393)
# windows (q, m) whose exp+rowsum runs on DVE instead of ACT
SCH_WINDOWS = {(0, 2), (0, 5), (1, 0), (1, 3), (1, 6)}

_NC_CACHE = {}


def _build_nc():
    nc = bacc.Bacc(
        "TRN2",
        target_bir_lowering=False,
        debug=False,
        enable_asserts=False,
        num_devices=N_CORES,
    )
    znt = nc.dram_tensor("znt", [P, 2, LCOLS], FP8, kind="ExternalInput")
    ident = nc.dram_tensor("ident", [P, P], BF16, kind="ExternalInput")
    onesb = nc.dram_tensor("onesb", [P, 1], BF16, kind="ExternalInput")
    rowsums_d = nc.dram_tensor("rowsums", [P, M_CHUNKS], F32,
                               kind="ExternalOutput")
    colsums_d = nc.dram_tensor("colsums", [1, CS_HI - CS_LO], F32,
                               kind="ExternalOutput")
    posv_d = nc.dram_tensor("posv", [P, M_CHUNKS], F32,
                            kind="ExternalOutput")

    with tile.TileContext(nc) as tc:
        with (
            tc.tile_pool(name="big", bufs=1) as big,
            tc.tile_pool(name="esbp", bufs=1) as esbp,
            tc.tile_pool(name="work", bufs=2) as work,
        ):
            znt_sb = big.tile([P, 2, LCOLS], FP8)
            # g0 heads the sync (HWDGE) ring so the first matmuls can
            # start early; id/ones ride the gpsimd ring behind g1
            nc.sync.dma_start(znt_sb[:, :, 0:1024], znt[:, :, 0:1024])
            nc.gpsimd.dma_start(znt_sb[:, :, 1024:2048],
                                znt[:, :, 1024:2048])
            id_sb = big.tile([P, P], BF16)
            nc.gpsimd.dma_start(id_sb[:], ident[:])
            ones_sb = big.tile([P, 1], BF16)
            nc.gpsimd.dma_start(ones_sb[:], onesb[:])
            nc.sync.dma_start(znt_sb[:, :, 2048:3072], znt[:, :, 2048:3072])
            nc.gpsimd.dma_start(znt_sb[:, :, 3072:4096],
                                znt[:, :, 3072:4096])
            nc.sync.dma_start(znt_sb[:, :, 4096:5120], znt[:, :, 4096:5120])

            # preload the Exp activation table while DMAs stream
            tbl = big.tile([P, 1], F32)
            nc.scalar.activation(tbl[:], ones_sb[:], AF.Exp)

            sums = big.tile([P, M_CHUNKS * N_WIN], F32)
            pos = big.tile([P, M_CHUNKS], F32)
            # exp windows for q=0,1 (cols 0:4096), bf16, m-major
            esb = [esbp.tile([P, M_CHUNKS, QCOL], BF16, tag=f"esb{q}",
                             name=f"esb{q}")
                   for q in range(2)]
            esum = [esbp.tile([P, 1024], BF16, tag="esum0", name="esum0"),
                    esbp.tile([P, QCOL], BF16, tag="esum1", name="esum1")]
            colsum_sb = big.tile([1, CS_HI - CS_LO], F32)
            d4buf = esbp.tile([P, M_CHUNKS, 1024], BF16, tag="d4b",
                              name="d4buf")

            tree_t = {}

            def tree_step(dst, src_m, sl, width, q, m):
                """Emit the bf16 adds that become ready once chunk m of
                src_m[:, :, sl] is written; after m=7, dst holds the full
                m-sum."""
                def t(tag):
                    if tag not in tree_t:
                        tree_t[tag] = work.tile([P, width], BF16, tag=tag,
                                                bufs=1, name=tag)
                    return tree_t[tag]
                if m % 2 == 1:
                    a = t(f"t{q}_0_{m - 1}")
                    nc.vector.tensor_tensor(a[:], src_m[:, m - 1, sl],
                                            src_m[:, m, sl], ALU.add)
                if m == 3:
                    b = t(f"t{q}_1_0")
                    nc.vector.tensor_tensor(b[:], t(f"t{q}_0_0")[:],
                                            t(f"t{q}_0_2")[:], ALU.add)
                if m == 7:
                    b = t(f"t{q}_1_4")
                    nc.vector.tensor_tensor(b[:], t(f"t{q}_0_4")[:],
                                            t(f"t{q}_0_6")[:], ALU.add)
                    nc.vector.tensor_tensor(dst, t(f"t{q}_1_0")[:],
                                            b[:], ALU.add)

            with tc.tile_pool(name="psum", bufs=2, space="PSUM") as psum_pool:
                def d4_window(m):
                    """d=4 window interleaved into the q stream: 2 MMs into
                    a half-used sim-tag tile, exp to SBUF d4buf, post-exp
                    diagonal extract (2x bf16 STT, no PSUM interlock)."""
                    ptf = psum_pool.tile([P, QCOL], F32, tag="sim",
                                         name="ptd4")
                    pt = ptf[:, 0:1024]
                    lhsT = znt_sb[:, :, m * P:(m + 1) * P]
                    for nn in range(2):
                        col = POS_Q0 + nn * NCOL
                        nc.tensor.matmul(
                            pt[:, nn * NCOL:(nn + 1) * NCOL],
                            lhsT,
                            znt_sb[:, :, col:col + NCOL],
                            start=True, stop=True, perf_mode=DROW)
                    col_ix = m * N_WIN + 2
                    nc.scalar.activation(
                        d4buf[:, m, :], pt[:], AF.Exp, scale=ACT_SCALE,
                        accum_out=sums[:, col_ix:col_ix + 1])
                    off = m * P
                    scr = work.tile([P, P], BF16, tag="extr")
                    nc.vector.scalar_tensor_tensor(
                        out=scr[:], in0=d4buf[:, m, off:off + P],
                        scalar=1.0, in1=id_sb[:],
                        op0=ALU.mult, op1=ALU.mult,
                        accum_out=pos[:, m:m + 1])

                d4_done = 0
                for q in range(2):      # cols [0:2048), [2048:4096) -> esb
                    for m in range(M_CHUNKS):
                        pt = psum_pool.tile([P, QCOL], F32, tag="sim")
                        lhsT = znt_sb[:, :, m * P:(m + 1) * P]
                        for nn in range(QCOL // NCOL):
                            col = q * QCOL + nn * NCOL
                            nc.tensor.matmul(
                                pt[:, nn * NCOL:(nn + 1) * NCOL],
                                lhsT,
                                znt_sb[:, :, col:col + NCOL],
                                start=True, stop=True, perf_mode=DROW)
                        col_ix = m * N_WIN + q
                        if (q, m) in SCH_WINDOWS:
                            # Schraudolph exp on DVE: int32(A*x + B) to an
                            # SBUF scratch (frees the PSUM bank), then read
                            # the bits as f32 -> bf16 esb + rowsum accum
                            iscr = work.tile([P, QCOL], I32, tag="sch",
                                             name="iscr")
                            nc.vector.tensor_scalar(
                                iscr[:], pt[:], SCH_A, SCH_B,
                                ALU.mult, ALU.add)
                            nc.vector.tensor_scalar(
                                esb[q][:, m, :], iscr[:].bitcast(F32),
                                1.0, 0.0, ALU.mult, ALU.add,
                                accum_out=sums[:, col_ix:col_ix + 1])
                        else:
                            nc.scalar.activation(
                                esb[q][:, m, :], pt[:], AF.Exp,
                                scale=ACT_SCALE,
                                accum_out=sums[:, col_ix:col_ix + 1])
                        # colsum m-tree, incrementally as chunks land
                        if q == 0:
                            tree_step(esum[0][:], esb[0], slice(1024, 2048),
                                      1024, 0, m)
                        else:
                            tree_step(esum[1][:], esb[1], slice(0, QCOL),
                                      QCOL, 1, m)
                        # a d4 window rides behind each DVE-offloaded
                        # window, feeding ACT during the DVE bubble
                        if (q, m) in SCH_WINDOWS and d4_done < 5:
                            d4_window(d4_done)
                            d4_done += 1
                for m in range(d4_done, M_CHUNKS):
                    d4_window(m)

            # colsum matmuls in a fresh pool once the sim ring retires
            with tc.tile_pool(name="psum2", bufs=2, space="PSUM") as pp2:
                for b in range(6):
                    c0 = b * NCOL            # offset within cols 1024:4096
                    if c0 < 1024:
                        src = esum[0][:, c0:c0 + NCOL]
                    else:
                        src = esum[1][:, c0 - 1024:c0 - 1024 + NCOL]
                    pc = pp2.tile([1, NCOL], F32, tag="cs", name="pc")
                    nc.tensor.matmul(pc[:], ones_sb[:], src,
                                     start=True, stop=True)
                    nc.vector.tensor_copy(colsum_sb[:, c0:c0 + NCOL], pc[:])
                nc.sync.dma_start(colsums_d[:], colsum_sb[:])

            # rowsum partials: sum the slots per m
            stot = big.tile([P, M_CHUNKS], F32)
            nc.vector.tensor_reduce(
                stot[:],
                sums[:].rearrange("p (m q) -> p m q", q=N_WIN),
                axis=mybir.AxisListType.X,
                op=ALU.add,
            )
            nc.sync.dma_start(rowsums_d[:], stot[:])
            nc.sync.dma_start(posv_d[:], pos[:])

    nc.compile()
    return nc


def _get_nc():
    if "nc" not in _NC_CACHE:
        _NC_CACHE["nc"] = _build_nc()
    return _NC_CACHE["nc"]


def _prepare_in_maps(z_i, z_j):
    import ml_dtypes

    z = np.concatenate(
        [np.asarray(z_i, np.float64), np.asarray(z_j, np.float64)], axis=0
    )
    zn = z / np.linalg.norm(z, axis=1, keepdims=True)
    q = (zn * QSCALE).astype(np.float32).astype(ml_dtypes.float8_e4m3)
    # znt[p, h, j] = q[j, h*128 + p]
    znt = np.ascontiguousarray(q.T.reshape(2, P, TWO_N).transpose(1, 0, 2))
    ident = np.eye(P, dtype=ml_dtypes.bfloat16)
    onesb = np.ones((P, 1), dtype=ml_dtypes.bfloat16)
    in_maps = []
    for c in range(N_CORES):
        zc = np.roll(znt, -ROWS_PER_CORE * c, axis=2)[:, :, :LCOLS]
        in_maps.append(
            {"znt": np.ascontiguousarray(zc), "ident": ident,
             "onesb": onesb})
    return in_maps


def _combine(results):
    """Assemble the loss from per-core rowsum/colsum/pos partials."""
    total = np.zeros(TWO_N, dtype=np.float64)
    posg = np.zeros(TWO_N, dtype=np.float64)
    for c in range(N_CORES):
        r0 = c * ROWS_PER_CORE
        rs = np.asarray(results[c]["rowsums"], np.float64)  # [128, 8]
        pv = np.asarray(results[c]["posv"], np.float64)
        for m in range(M_CHUNKS):
            gsl = slice(r0 + m * P, r0 + (m + 1) * P)
            total[gsl] += rs[:, m]
            posg[gsl] = pv[:, m]
        cs = np.asarray(results[c]["colsums"], np.float64).ravel()  # [3072]
        gidx = (r0 + CS_LO + np.arange(CS_HI - CS_LO)) % TWO_N
        np.add.at(total, gidx, cs)
    denom = total - np.exp(TEMP_SCALE)
    terms = np.log(denom) - np.log(posg)
    return float(terms.mean())


def kernel(z_i, z_j):
    nc = _get_nc()
    in_maps = _prepare_in_maps(z_i, z_j)
    res = run_bass_kernel_spmd(nc, in_maps, core_ids=list(range(N_CORES)))
    return np.float32(_combine(res.results))


if __name__ == "__main__":
    rng = np.random.default_rng(0)
    z_i = rng.standard_normal((4096, 256), dtype=np.float32)
    z_j = rng.standard_normal((4096, 256), dtype=np.float32)
    print("loss:", kernel(z_i, z_j))


# revision 26
# speedup vs baseline: 1.0222x; 1.0222x over previous
"""NT-Xent loss on 8 Trainium2 NeuronCores — fp8 + symmetric 5/8 scheme.

Math: z = concat(z_i, z_j) [8192, 256]; E = exp(2*cos_sim(z)) is
symmetric, so row sums = column sums. Blocked over 8 superblocks of
1024 rows, core c owns rows [1024c, 1024c+1024) and computes sim
against column superblocks d = 0..4 (its own + the next four, mod 8).
d in {1,2,3} blocks are mirrored via COLUMN sums (row sums of the
uncomputed transpose block); the d=4 block is computed by both members
of each (c, c+4) pair, so no mirror is needed there. Every global row
then receives exactly 8 superblock contributions: 5 direct row-sum
partials from its owner core + 3 column-sum partials from cores
c-1, c-2, c-3. The host assembles denom = sum - e^2, takes ln in
fp64, and averages — O(2N) work.

Per core (identical SPMD program on column-rotated inputs):
  - Host normalizes rows in fp64, scales by 16, quantizes to fp8 e4m3,
    ships the transposed K-packed layout znt[p, h, j] = q[j, h*128+p]
    ([128, 2, 5120], rotated by -1024c cols: local cols 0:5120 =
    superblocks c..c+4, local rows = local cols 0:1024).
  - Sim matmuls: fp8 DoubleRow, one matmul per [128, 512] output
    covering the full K=256 contraction. PSUM = 256*sim.
  - ACT exps [128, 2048] PSUM windows (scale 1/128) with fused row-sum
    accumulation, writing exp as bf16 to SBUF (esb) for the colsum
    path. FIVE of the 16 q-windows are offloaded to DVE via the
    Schraudolph bit-trick exp (int32(A*x+B) then read the bits as f32;
    ~3% per-element error that washes out in the row sums), balancing
    ACT (the pacer) against DVE idle time. The [4096:5120) d=4 window
    is exp'd in place on ACT and carries the positive-pair diagonal
    (extracted pre-exp by a DVE identity-mask reduce).
  - Colsums over cols 1024:4096: incremental DVE bf16 tree-sum over
    the 8 m-chunks (2x mode), then one ones-matmul per 512-col block
    -> [1, 512] PSUM -> DVE copy -> colsum_sb, spread across the d4
    phase.
  - Outputs per core: rowsums [128, 8], colsums [1, 3072], posv
    [128, 8] (scaled sim, x1/128 on host).

Measured pitfalls baked in: DVE tensor_reduce has no 2x uop (use ACT
accum_out for rowsums); gpsimd partition_all_reduce is ~5x slower than
its cost model (use ones-matmuls); fp8 DoubleRow streams ~0.5-0.6
ns/moving-row (not the 2x the cost model claims); ACT costs ~0.95
ns/col on these windows regardless of destination.

fp8 error budget: e4m3 sigma~3.6%/elem -> sim noise sigma~3e-3 which
averages out in row sums and the 8192-row mean; Schraudolph adds ~3%
per-element on 5/16 of the denominator mass -> ~1e-4 loss error.
"""

import os
import sys

sys.path.insert(0, "/opt/trn_rl_repo")
os.environ.setdefault("MYCRO_LOCAL_CACHE", "1")

import numpy as np

import concourse.bass as bass
import concourse.mybir as mybir
from concourse import bacc, tile
from concourse.bass_utils import run_bass_kernel_spmd

F32 = mybir.dt.float32
BF16 = mybir.dt.bfloat16
FP8 = mybir.dt.float8e4
I32 = mybir.dt.int32
AF = mybir.ActivationFunctionType
ALU = mybir.AluOpType
DROW = mybir.MatmulPerfMode.DoubleRow

N_CORES = 8
TWO_N = 8192
D = 256
P = 128
ROWS_PER_CORE = TWO_N // N_CORES  # 1024
M_CHUNKS = ROWS_PER_CORE // P     # 8 local row chunks
NCOL = 512                        # matmul free dim (one PSUM bank)
QCOL = 2048                       # ACT window for q=0,1
LCOLS = 5 * ROWS_PER_CORE         # 5120 local cols (superblocks d=0..4)
CS_LO, CS_HI = 1024, 4096         # colsum region (d=1..3)
POS_Q0 = 4096                     # start of the d=4 window
N_WIN = 3                         # rowsum slots per m: q0, q1, d4
TEMP_SCALE = 2.0
QSCALE = 16.0                     # host fp8 quant scale; psum = 256*sim
ACT_SCALE = TEMP_SCALE / (QSCALE * QSCALE)   # exp(psum/128)
POS_SCALE = TEMP_SCALE / (QSCALE * QSCALE)

# Schraudolph: exp(x) ~ bits_as_f32(int32(2^23/ln2 * x + 127*2^23 - C))
SCH_A = ACT_SCALE * (1 << 23) / float(np.log(2.0))   # applied to psum
SCH_B = float(127 * (1 << 23) - 366393)
# windows (q, m) whose exp+rowsum runs on DVE instead of ACT
SCH_WINDOWS = {(0, 2), (0, 5), (1, 0), (1, 3), (1, 6)}

_NC_CACHE = {}


def _build_nc():
    nc = bacc.Bacc(
        "TRN2",
        target_bir_lowering=False,
        debug=False,
        enable_asserts=False,
        num_devices=N_CORES,
    )
    znt = nc.dram_tensor("znt", [P, 2, LCOLS], FP8, kind="ExternalInput")
    ident = nc.dram_tensor("ident", [P, P], BF16, kind="ExternalInput")
    onesb = nc.dram_tensor("onesb", [P, 1], BF16, kind="ExternalInput")
    rowsums_d = nc.dram_tensor("rowsums", [P, M_CHUNKS], F32,
                               kind="ExternalOutput")
    colsums_d = nc.dram_tensor("colsums", [1, CS_HI - CS_LO], F32,
                               kind="ExternalOutput")
    posv_d = nc.dram_tensor("posv", [P, M_CHUNKS], F32,
                            kind="ExternalOutput")

    with tile.TileContext(nc) as tc:
        with (
            tc.tile_pool(name="big", bufs=1) as big,
            tc.tile_pool(name="esbp", bufs=1) as esbp,
            tc.tile_pool(name="work", bufs=2) as work,
        ):
            znt_sb = big.tile([P, 2, LCOLS], FP8)
            # g0 heads the sync (HWDGE) ring so the first matmuls can
            # start early; id/ones ride the gpsimd ring behind g1
            nc.sync.dma_start(znt_sb[:, :, 0:1024], znt[:, :, 0:1024])
            nc.gpsimd.dma_start(znt_sb[:, :, 1024:2048],
                                znt[:, :, 1024:2048])
            id_sb = big.tile([P, P], BF16)
            nc.gpsimd.dma_start(id_sb[:], ident[:])
            ones_sb = big.tile([P, 1], BF16)
            nc.gpsimd.dma_start(ones_sb[:], onesb[:])
            nc.sync.dma_start(znt_sb[:, :, 2048:3072], znt[:, :, 2048:3072])
            nc.gpsimd.dma_start(znt_sb[:, :, 3072:4096],
                                znt[:, :, 3072:4096])
            nc.sync.dma_start(znt_sb[:, :, 4096:5120], znt[:, :, 4096:5120])

            # preload the Exp activation table while DMAs stream
            tbl = big.tile([P, 1], F32)
            nc.scalar.activation(tbl[:], ones_sb[:], AF.Exp)

            sums = big.tile([P, M_CHUNKS * N_WIN], F32)
            pos = big.tile([P, M_CHUNKS], F32)
            # exp windows for q=0,1 (cols 0:4096), bf16, m-major
            esb = [esbp.tile([P, M_CHUNKS, QCOL], BF16, tag=f"esb{q}",
                             name=f"esb{q}")
                   for q in range(2)]
            esum = [esbp.tile([P, 1024], BF16, tag="esum0", name="esum0"),
                    esbp.tile([P, QCOL], BF16, tag="esum1", name="esum1")]
            colsum_sb = big.tile([1, CS_HI - CS_LO], F32)
            d4buf = esbp.tile([P, M_CHUNKS, 1024], BF16, tag="d4b",
                              name="d4buf")

            tree_t = {}

            def tree_step(dst, src_m, sl, width, q, m):
                """Emit the bf16 adds that become ready once chunk m of
                src_m[:, :, sl] is written; after m=7, dst holds the full
                m-sum."""
                def t(tag):
                    if tag not in tree_t:
                        tree_t[tag] = work.tile([P, width], BF16, tag=tag,
                                                bufs=1, name=tag)
                    return tree_t[tag]
                if m % 2 == 1:
                    a = t(f"t{q}_0_{m - 1}")
                    nc.vector.tensor_tensor(a[:], src_m[:, m - 1, sl],
                                            src_m[:, m, sl], ALU.add)
                if m == 3:
                    b = t(f"t{q}_1_0")
                    nc.vector.tensor_tensor(b[:], t(f"t{q}_0_0")[:],
                                            t(f"t{q}_0_2")[:], ALU.add)
                if m == 7:
                    b = t(f"t{q}_1_4")
                    nc.vector.tensor_tensor(b[:], t(f"t{q}_0_4")[:],
                                            t(f"t{q}_0_6")[:], ALU.add)
                    nc.vector.tensor_tensor(dst, t(f"t{q}_1_0")[:],
                                            b[:], ALU.add)

            with tc.tile_pool(name="psum", bufs=2, space="PSUM") as psum_pool:
                def d4_window(m):
                    """d=4 window interleaved into the q stream: 2 MMs into
                    a half-used sim-tag tile, exp to SBUF d4buf, post-exp
                    diagonal extract (2x bf16 STT, no PSUM interlock)."""
                    ptf = psum_pool.tile([P, QCOL], F32, tag="sim",
                                         name="ptd4")
                    pt = ptf[:, 0:1024]
                    lhsT = znt_sb[:, :, m * P:(m + 1) * P]
                    for nn in range(2):
                        col = POS_Q0 + nn * NCOL
                        nc.tensor.matmul(
                            pt[:, nn * NCOL:(nn + 1) * NCOL],
                            lhsT,
                            znt_sb[:, :, col:col + NCOL],
                            start=True, stop=True, perf_mode=DROW)
                    col_ix = m * N_WIN + 2
                    nc.scalar.activation(
                        d4buf[:, m, :], pt[:], AF.Exp, scale=ACT_SCALE,
                        accum_out=sums[:, col_ix:col_ix + 1])
                    off = m * P
                    scr = work.tile([P, P], BF16, tag="extr")
                    nc.vector.scalar_tensor_tensor(
                        out=scr[:], in0=d4buf[:, m, off:off + P],
                        scalar=1.0, in1=id_sb[:],
                        op0=ALU.mult, op1=ALU.mult,
                        accum_out=pos[:, m:m + 1])

                d4_done = 0
                for q in range(2):      # cols [0:2048), [2048:4096) -> esb
                    for m in range(M_CHUNKS):
                        pt = psum_pool.tile([P, QCOL], F32, tag="sim")
                        lhsT = znt_sb[:, :, m * P:(m + 1) * P]
                        for nn in range(QCOL // NCOL):
                            col = q * QCOL + nn * NCOL
                            nc.tensor.matmul(
                                pt[:, nn * NCOL:(nn + 1) * NCOL],
                                lhsT,
                                znt_sb[:, :, col:col + NCOL],
                                start=True, stop=True, perf_mode=DROW)
                        col_ix = m * N_WIN + q
                        if (q, m) in SCH_WINDOWS:
                            # Schraudolph exp on DVE: int32(A*x + B) to an
                            # SBUF scratch (frees the PSUM bank), then read
                            # the bits as f32 -> bf16 esb + rowsum accum
                            iscr = work.tile([P, QCOL], I32, tag="sch",
                                             name="iscr")
                            nc.vector.tensor_scalar(
                                iscr[:], pt[:], SCH_A, SCH_B,
                                ALU.mult, ALU.add)
                            nc.vector.tensor_scalar(
                                esb[q][:, m, :], iscr[:].bitcast(F32),
                                1.0, 0.0, ALU.mult, ALU.add,
                                accum_out=sums[:, col_ix:col_ix + 1])
                        else:
                            nc.scalar.activation(
                                esb[q][:, m, :], pt[:], AF.Exp,
                                scale=ACT_SCALE,
                                accum_out=sums[:, col_ix:col_ix + 1])
                        # colsum m-tree, incrementally as chunks land
                        if q == 0:
                            tree_step(esum[0][:], esb[0], slice(1024, 2048),
                                      1024, 0, m)
                        else:
                            tree_step(esum[1][:], esb[1], slice(0, QCOL),
                                      QCOL, 1, m)
                        # a d4 window rides behind each DVE-offloaded
                        # window, feeding ACT during the DVE bubble
                        if (q, m) in SCH_WINDOWS and d4_done < 5:
                            d4_window(d4_done)
                            d4_done += 1
                for m in range(d4_done, M_CHUNKS):
                    d4_window(m)

            # colsum matmuls in a fresh pool once the sim ring retires
            with tc.tile_pool(name="psum2", bufs=2, space="PSUM") as pp2:
                for b in range(6):
                    c0 = b * NCOL            # offset within cols 1024:4096
                    if c0 < 1024:
                        src = esum[0][:, c0:c0 + NCOL]
                    else:
                        src = esum[1][:, c0 - 1024:c0 - 1024 + NCOL]
                    pc = pp2.tile([1, NCOL], F32, tag="cs", name="pc")
                    nc.tensor.matmul(pc[:], ones_sb[:], src,
                                     start=True, stop=True)
                    # alternate the PSUM->SBUF copies between DVE and the
                    # scalar engine (idle after its last window) to halve
                    # the single-partition copy tail
                    if b % 2 == 0:
                        nc.vector.tensor_copy(colsum_sb[:, c0:c0 + NCOL],
                                              pc[:])
                    else:
                        nc.scalar.copy(colsum_sb[:, c0:c0 + NCOL], pc[:])
                nc.sync.dma_start(colsums_d[:], colsum_sb[:])

            # rowsum partials: sum the slots per m
            stot = big.tile([P, M_CHUNKS], F32)
            nc.vector.tensor_reduce(
                stot[:],
                sums[:].rearrange("p (m q) -> p m q", q=N_WIN),
                axis=mybir.AxisListType.X,
                op=ALU.add,
            )
            nc.sync.dma_start(rowsums_d[:], stot[:])
            nc.sync.dma_start(posv_d[:], pos[:])

    nc.compile()
    return nc


def _get_nc():
    if "nc" not in _NC_CACHE:
        _NC_CACHE["nc"] = _build_nc()
    return _NC_CACHE["nc"]


def _prepare_in_maps(z_i, z_j):
    import ml_dtypes

    z = np.concatenate(
        [np.asarray(z_i, np.float64), np.asarray(z_j, np.float64)], axis=0
    )
    zn = z / np.linalg.norm(z, axis=1, keepdims=True)
    q = (zn * QSCALE).astype(np.float32).astype(ml_dtypes.float8_e4m3)
    # znt[p, h, j] = q[j, h*128 + p]
    znt = np.ascontiguousarray(q.T.reshape(2, P, TWO_N).transpose(1, 0, 2))
    ident = np.eye(P, dtype=ml_dtypes.bfloat16)
    onesb = np.ones((P, 1), dtype=ml_dtypes.bfloat16)
    in_maps = []
    for c in range(N_CORES):
        zc = np.roll(znt, -ROWS_PER_CORE * c, axis=2)[:, :, :LCOLS]
        in_maps.append(
            {"znt": np.ascontiguousarray(zc), "ident": ident,
             "onesb": onesb})
    return in_maps


def _combine(results):
    """Assemble the loss from per-core rowsum/colsum/pos partials."""
    total = np.zeros(TWO_N, dtype=np.float64)
    posg = np.zeros(TWO_N, dtype=np.float64)
    for c in range(N_CORES):
        r0 = c * ROWS_PER_CORE
        rs = np.asarray(results[c]["rowsums"], np.float64)  # [128, 8]
        pv = np.asarray(results[c]["posv"], np.float64)
        for m in range(M_CHUNKS):
            gsl = slice(r0 + m * P, r0 + (m + 1) * P)
            total[gsl] += rs[:, m]
            posg[gsl] = pv[:, m]
        cs = np.asarray(results[c]["colsums"], np.float64).ravel()  # [3072]
        gidx = (r0 + CS_LO + np.arange(CS_HI - CS_LO)) % TWO_N
        np.add.at(total, gidx, cs)
    denom = total - np.exp(TEMP_SCALE)
    terms = np.log(denom) - np.log(posg)
    return float(terms.mean())


def kernel(z_i, z_j):
    nc = _get_nc()
    in_maps = _prepare_in_maps(z_i, z_j)
    res = run_bass_kernel_spmd(nc, in_maps, core_ids=list(range(N_CORES)))
    return np.float32(_combine(res.results))


if __name__ == "__main__":
    rng = np.random.default_rng(0)
    z_i = rng.standard_normal((4096, 256), dtype=np.float32)
    z_j = rng.standard_normal((4096, 256), dtype=np.float32)
    print("loss:", kernel(z_i, z_j))
